# revision 41
# baseline (speedup 1.0000x reference)
"""Trainium2 Bass kernel for nn_Coords2Eps (scatter of per-atom Gaussians into a
128^3 grid, then eps = exp(-rho)*(EPS_OUT-EPS_IN)+EPS_IN).

Strategy (pure data-parallel over batch, 8 cores):
  - Each core owns one batch: its atoms + full 128^3 grid.
  - The (x, y) plane is tiled into 128 buckets of 8x16 voxels. Each atom's
    8x8x8 separable Gaussian window overlaps 1-4 buckets -> one "entry" per
    overlapped bucket. Every bucket's entries start at a fresh 128-partition
    tile (consecutive matmuls with different partition bases crash the device).
  - On device, per entry (batched broadcast-AP vector ops, fp32 intermediates):
      gz[e, z]   = exp(-((z - za) s)^2) masked to the 8-wide z window
      patch[e,p] = gx[e,dx] * gy[e,dy]   (masked 8x16 window patch)
    Window mask folded via t2 = max(t, (10 v)^2 - 1225), v = coord - center.
  - Dense z restricted to [zlo, zhi) covering all windows; constant z-strips
    are memset in the staging tile.
  - Per bucket, PE matmuls (fp32 operands): rho[p, z] = patch.T @ gz in PSUM;
    epilogue exp(-rho)*72.5 + 6.5 on ACT/DVE.
  - Output is written bucket-major (nonempty buckets first) so each 4-bucket
    group is ONE contiguous DMA and all empty buckets are a few big constant
    fills; the host inverts the permutation (pure layout transform).

kernel(**inputs) takes the FULL inputs and returns the FULL output.
"""
import sys
import numpy as np

sys.path.insert(0, "/opt/trn_rl_repo")

import concourse.bacc as bacc
import concourse.tile as tile
import concourse.mybir as mybir
from concourse.bass_utils import run_bass_kernel_spmd

F32 = mybir.dt.float32
F16 = mybir.dt.float16
I32 = mybir.dt.int32
AF = mybir.ActivationFunctionType
ALU = mybir.AluOpType

BOX = 128
W = 8
EPS_IN = 6.5
EPS_OUT = 79.0
LN_SCALE = float(np.log(EPS_OUT - EPS_IN))  # ln(72.5)

TX, TY = 8, 16
NBX, NBY = BOX // TX, BOX // TY     # 16, 8
NBUCKET = NBX * NBY                 # 128
DEAD_F = 1000.0
CUSHION = 4
NCORES = 8
MM_DT = F32   # PE fp32 (emulated): rel err 1.4e-5 vs 6.3e-4 for fp16, ~same time
PARAM_KEYS = ("ZA", "S", "CZ", "RX", "CX", "RY", "CY")
NPK = len(PARAM_KEYS)


# ----------------------------------------------------------------- host side

def _bucket_counts(coords_b, num_atoms_b):
    n = int(num_atoms_b)
    xyz = coords_b.reshape(-1, 3)[:n]
    base = np.floor(xyz).astype(np.int64) - (W // 2 - 1)
    bx, by = base[:, 0], base[:, 1]
    jx0, jx1 = bx // TX, (bx + W - 1) // TX
    jy0, jy1 = by // TY, (by + W - 1) // TY
    cnt = np.zeros(NBUCKET, np.int64)
    np.add.at(cnt, jx0 * NBY + jy0, 1)
    m = jx1 != jx0
    np.add.at(cnt, jx1[m] * NBY + jy0[m], 1)
    m2 = jy1 != jy0
    np.add.at(cnt, jx0[m2] * NBY + jy1[m2], 1)
    m3 = m & m2
    np.add.at(cnt, jx1[m3] * NBY + jy1[m3], 1)
    return cnt


def _make_layout(units):
    """Every bucket starts at a tile (128-partition) boundary; all matmul
    chunks contract partitions [0, plen)."""
    starts = np.zeros(NBUCKET, np.int64)
    tot = 0
    for b in range(NBUCKET):
        starts[b] = tot
        tot += 4 * ((int(units[b]) + 3) // 4)
    nt = max(1, (tot + 3) // 4)
    chunks_by_bucket = {}
    for b in range(NBUCKET):
        if units[b] == 0:
            continue
        t0 = int(starts[b]) // 4
        rem = int(units[b])
        segs = []
        t = t0
        while rem > 0:
            take = min(rem, 4)
            segs.append((t, 0, take * 32))
            t += 1
            rem -= take
        chunks_by_bucket[b] = segs
    return nt, starts, chunks_by_bucket


def _host_prep(coords_b, sigma_b, num_atoms_b, units, starts, nt):
    """Single [128, 7*nt] f32 param image (concatenated PARAM_KEYS)."""
    n = int(num_atoms_b)
    xyz = coords_b.reshape(-1, 3)[:n].astype(np.float64)
    sig = sigma_b[:n].astype(np.float64)
    xa, ya, za = xyz[:, 0], xyz[:, 1], xyz[:, 2]
    base = np.floor(xyz).astype(np.int64) - (W // 2 - 1)
    bx, by, bz = base[:, 0], base[:, 1], base[:, 2]
    s = np.sqrt(0.5) / sig
    assert (base >= 0).all() and (base + W <= BOX).all(), "window out of bounds"

    params = {k: np.zeros((128, nt), np.float32) for k in PARAM_KEYS}
    params["S"][:] = 1.0
    for k in ("CZ", "CX", "CY"):
        params[k][:] = -DEAD_F

    jx0, jx1 = bx // TX, (bx + W - 1) // TX
    jy0, jy1 = by // TY, (by + W - 1) // TY
    counts = np.zeros(NBUCKET, np.int64)
    for a in range(n):
        for jx in {int(jx0[a]), int(jx1[a])}:
            for jy in {int(jy0[a]), int(jy1[a])}:
                b = jx * NBY + jy
                i = counts[b]
                assert i < units[b] * 32, f"bucket {b} overflow"
                counts[b] += 1
                g = starts[b] * 32 + i
                t, p = g // 128, g % 128
                rx = xa[a] - TX * jx
                ry = ya[a] - TY * jy
                params["ZA"][p, t] = za[a]
                params["S"][p, t] = s[a]
                params["CZ"][p, t] = bz[a] + 3.5
                params["RX"][p, t] = rx
                params["CX"][p, t] = (bx[a] - TX * jx) + 3.5
                params["RY"][p, t] = ry
                params["CY"][p, t] = (by[a] - TY * jy) + 3.5
    return np.concatenate([params[k] for k in PARAM_KEYS], axis=1)


# --------------------------------------------------------------- device side

def _emit_build(nc, pool, iota_f, par, nt_c, c0, width, rkey, fkey,
                out_tile, out_cols, t2_pool=False):
    """Masked-Gaussian rows -> out_tile[:, out_cols] (fp16).

    par(key, c0, nt_c) -> broadcast AP. Engine split: d0/d/t2 on DVE,
    v on Pool(gpsimd), t/u/exp on ACT.
    """
    ncol = nt_c * width
    sh3 = [128, nt_c, width]

    def b_iota():
        return iota_f[:].rearrange("p (o w) -> p o w", o=1).broadcast_to(sh3)

    def r3(tl):
        return tl[:].rearrange("p (t w) -> p t w", w=width)

    d0 = pool.tile([128, ncol], F32, tag=f"d0_{width}")
    nc.vector.tensor_tensor(r3(d0), b_iota(), par(rkey, c0, nt_c),
                            op=ALU.subtract)
    d = pool.tile([128, ncol], F32, tag=f"d_{width}")
    nc.vector.tensor_tensor(r3(d), r3(d0), par("S", c0, nt_c), op=ALU.mult)
    t = pool.tile([128, ncol], F32, tag=f"t_{width}")
    nc.scalar.activation(t[:], d[:], AF.Square)
    v = pool.tile([128, ncol], F32, tag=f"v_{width}")
    nc.gpsimd.tensor_tensor(r3(v), b_iota(), par(fkey, c0, nt_c),
                            op=ALU.subtract)
    u = pool.tile([128, ncol], F32, tag=f"u_{width}")
    nc.scalar.activation(u[:], v[:], AF.Square, scale=10.0)
    t2 = pool.tile([128, ncol], F32, tag=f"t2_{width}")
    t2eng = nc.gpsimd if t2_pool else nc.vector
    t2eng.scalar_tensor_tensor(
        t2[:], u[:], -1225.0, t[:], op0=ALU.add, op1=ALU.max)
    nc.scalar.activation(out_tile[:, out_cols], t2[:], AF.Exp, scale=-1.0)


def _build_nc(nt, chunks_by_bucket, nonempty, empty, zlo, zhi):
    from contextlib import ExitStack
    zw = zhi - zlo
    nne = len(nonempty)
    nc = bacc.Bacc("TRN2", target_bir_lowering=False, debug=False,
                   num_devices=NCORES)
    param_d = nc.dram_tensor("PARAMS", [128, NPK * nt], F32,
                             kind="ExternalInput")
    eps_d = nc.dram_tensor("eps", [NBUCKET * 128 * BOX], F32,
                           kind="ExternalOutput")
    # bucket-major, rank order = nonempty + empty; [p, rank, z] iteration view
    eps_prz = eps_d.ap().rearrange("(b p z) -> p b z", b=NBUCKET, p=128, z=BOX)

    NCH = max(1, (nt + 5) // 6)
    bounds = [round(i * nt / NCH) for i in range(NCH + 1)]

    with tile.TileContext(nc) as tc, ExitStack() as ctx:
        const = ctx.enter_context(tc.tile_pool(name="const", bufs=1))
        big = ctx.enter_context(tc.tile_pool(name="big", bufs=1))
        scratch = ctx.enter_context(tc.tile_pool(name="scratch", bufs=5))
        xyscr = ctx.enter_context(tc.tile_pool(name="xyscr", bufs=1))
        psum = ctx.enter_context(tc.tile_pool(name="psum", bufs=6, space="PSUM"))
        epsp = ctx.enter_context(tc.tile_pool(name="epsp", bufs=6))

        # ---- constants
        iotas = {}
        for width, base in ((TX, 0), (TY, 0), (zw, zlo)):
            ii = const.tile([128, width], I32, tag=f"ii_{width}")
            nc.gpsimd.iota(ii[:], pattern=[[1, width]], base=base,
                           channel_multiplier=0)
            iof = const.tile([128, width], F32, tag=f"iof_{width}")
            nc.vector.tensor_copy(iof[:], ii[:])
            iotas[width] = iof
        c79 = const.tile([128, 2048], F32)
        nc.vector.memset(c79[:], EPS_OUT)
        bias_ln = const.tile([128, 1], F32)
        nc.vector.memset(bias_ln[:], LN_SCALE)

        # ---- params (one DMA)
        par_t = const.tile([128, NPK * nt], F32)
        nc.sync.dma_start(par_t[:], param_d.ap())
        pk_off = {k: i * nt for i, k in enumerate(PARAM_KEYS)}

        def mkpar(width):
            def par(key, c0, nt_c):
                o = pk_off[key] + c0
                return (par_t[:, o:o + nt_c]
                        .rearrange("p (t o) -> p t o", o=1)
                        .broadcast_to([128, nt_c, width]))
            return par

        # ---- persistent matmul operands (fp16)
        gz_t = big.tile([128, nt * zw], MM_DT, tag="gz")
        patch_t = big.tile([128, nt * TX * TY], MM_DT, tag="patch")
        gxm = big.tile([128, nt * TX], MM_DT, tag="gxm")
        gym = big.tile([128, nt * TY], MM_DT, tag="gym")

        # ---- constant fill for the empty-bucket tail (ranks nne..NBUCKET)
        FILLW = 16  # slots per fill DMA (c79 holds 16*128 elems per partition)
        r = nne
        while r < NBUCKET:
            rn = min(FILLW, NBUCKET - r)
            nc.sync.dma_start(eps_prz[:, r:r + rn, :],
                              c79[:, :rn * BOX]
                              .rearrange("p (b z) -> p b z", z=BOX))
            r += rn

        # ---- x/y window builds (small; unchunked)
        _emit_build(nc, xyscr, iotas[TX], mkpar(TX), nt, 0, TX,
                    "RX", "CX", gxm, slice(0, nt * TX))
        _emit_build(nc, xyscr, iotas[TY], mkpar(TY), nt, 0, TY,
                    "RY", "CY", gym, slice(0, nt * TY))

        # ---- per-group matmul + epilogue emitter
        def emit_group(grp, rank):
            ng = len(grp)
            acc = psum.tile([128, 512], F32)
            for q, b in enumerate(grp):
                oc = slice(q * BOX + zlo, q * BOX + zhi)
                nseg = len(chunks_by_bucket[b])
                for i, (t, plo, plen) in enumerate(chunks_by_bucket[b]):
                    nc.tensor.matmul(
                        acc[:, oc],
                        patch_t[plo:plo + plen, t * TX * TY:(t + 1) * TX * TY],
                        gz_t[plo:plo + plen, t * zw:(t + 1) * zw],
                        start=(i == 0), stop=(i == nseg - 1))
            ep = epsp.tile([128, 512], F32)
            epv = ep[:].rearrange("p (q z) -> p q z", z=BOX)
            accv = acc[:].rearrange("p (q z) -> p q z", z=BOX)
            if zlo > 0:
                nc.gpsimd.memset(epv[:, :ng, 0:zlo], EPS_OUT)
            if zhi < BOX:
                nc.gpsimd.memset(epv[:, :ng, zhi:BOX], EPS_OUT)
            nc.scalar.activation(epv[:, :ng, zlo:zhi], accv[:, :ng, zlo:zhi],
                                 AF.Exp, bias=bias_ln[:], scale=-1.0)
            nc.vector.tensor_scalar_add(epv[:, :ng, zlo:zhi],
                                        epv[:, :ng, zlo:zhi], EPS_IN)
            nc.sync.dma_start(eps_prz[:, rank:rank + ng, :],
                              epv[:, :ng, :])

        groups = [nonempty[i:i + 4] for i in range(0, len(nonempty), 4)]
        glast = [max(t + 1 for b in grp for (t, _, _) in chunks_by_bucket[b])
                 for grp in groups]
        granks = list(np.cumsum([0] + [len(g) for g in groups[:-1]]))

        # ---- z build + patch outer, chunked; groups interleave as their
        #      tiles complete
        gidx = 0
        for c in range(NCH):
            c0, c1 = bounds[c], bounds[c + 1]
            nt_c = c1 - c0
            if nt_c == 0:
                continue
            _emit_build(nc, scratch, iotas[zw], mkpar(zw), nt_c, c0, zw,
                        "ZA", "CZ", gz_t, slice(c0 * zw, c1 * zw))
            sh4 = [128, nt_c, TX, TY]
            gx_b = (gxm[:, c0 * TX:c1 * TX]
                    .rearrange("p (t x o) -> p t x o", x=TX, o=1)
                    .broadcast_to(sh4))
            gy_b = (gym[:, c0 * TY:c1 * TY]
                    .rearrange("p (t o y) -> p t o y", o=1, y=TY)
                    .broadcast_to(sh4))
            eng = nc.gpsimd if (c % 2 == 1) else nc.vector
            eng.tensor_tensor(
                patch_t[:, c0 * TX * TY:c1 * TX * TY]
                .rearrange("p (t x y) -> p t x y", x=TX, y=TY),
                gx_b, gy_b, op=ALU.mult)
            while gidx < len(groups) and glast[gidx] <= c1:
                emit_group(groups[gidx], int(granks[gidx]))
                gidx += 1
        while gidx < len(groups):
            emit_group(groups[gidx], int(granks[gidx]))
            gidx += 1
    nc.compile()
    return nc


# ------------------------------------------------------------------- driver

_CACHE = {}


def _layout_from_inputs(coords, num_atoms):
    B = coords.shape[0]
    all_counts = np.stack([
        _bucket_counts(coords[b], num_atoms[b]) for b in range(B)])
    mx = all_counts.max(axis=0)
    units = np.where(mx > 0, (mx + CUSHION + 31) // 32, 0)
    nt, starts, chunks_by_bucket = _make_layout(units)
    nonempty = [b for b in range(NBUCKET) if units[b] > 0]
    empty = [b for b in range(NBUCKET) if units[b] == 0]
    zlo, zhi = BOX, 0
    for b in range(B):
        n = int(num_atoms[b])
        z = coords[b].reshape(-1, 3)[:n, 2]
        bz = np.floor(z).astype(np.int64) - (W // 2 - 1)
        zlo = min(zlo, int(bz.min()))
        zhi = max(zhi, int(bz.max()) + W)
    zlo = max(0, (zlo // 4) * 4)
    zhi = min(BOX, ((zhi + 3) // 4) * 4)
    return units, nt, starts, chunks_by_bucket, nonempty, empty, zlo, zhi


def kernel(coords, assigned_params, num_atoms):
    coords = np.asarray(coords, dtype=np.float32)
    assigned_params = np.asarray(assigned_params, dtype=np.float32)
    num_atoms = np.asarray(num_atoms, dtype=np.int32)
    B = coords.shape[0]
    assert B == NCORES, f"expected {NCORES} batches, got {B}"

    (units, nt, starts, cbb, nonempty, empty, zlo,
     zhi) = _layout_from_inputs(coords, num_atoms)

    in_maps = []
    for b in range(B):
        img = _host_prep(coords[b], assigned_params[b, :, 1], num_atoms[b],
                         units, starts, nt)
        in_maps.append({"PARAMS": img})

    key = (tuple(units.tolist()), zlo, zhi)
    if key not in _CACHE:
        _CACHE[key] = _build_nc(nt, cbb, nonempty, empty, zlo, zhi)
    nc = _CACHE[key]
    res = run_bass_kernel_spmd(nc, in_maps, list(range(NCORES)))

    # invert the bucket-rank permutation (pure layout transform)
    order = nonempty + empty           # rank -> bucket id
    inv = np.empty(NBUCKET, np.int64)
    inv[np.array(order)] = np.arange(NBUCKET)
    out = np.empty((B, BOX, BOX, BOX), np.float32)
    for i in range(B):
        raw = res.results[i]["eps"].reshape(NBUCKET, 128, BOX)
        byb = raw[inv]                                 # [bucket, (dx dy), z]
        out[i] = (byb.reshape(NBX, NBY, TX, TY, BOX)
                  .transpose(0, 2, 1, 3, 4)
                  .reshape(BOX, BOX, BOX))
    return out


if __name__ == "__main__":
    rng = np.random.default_rng(0)
    coords = rng.uniform(0.2 * BOX, 0.8 * BOX, (8, 3 * 2048)).astype(np.float32)
    ap = np.stack([np.ones((8, 2048), np.float32),
                   1.0 + rng.random((8, 2048), dtype=np.float32)], axis=-1)
    na = np.full((8,), 2048, np.int32)
    out = kernel(coords=coords, assigned_params=ap, num_atoms=na)
    print("out", out.shape, out.dtype, out.min(), out.max())


# revision 42
# speedup vs baseline: 1.0250x; 1.0250x over previous
"""Trainium2 Bass kernel for nn_Coords2Eps (scatter of per-atom Gaussians into a
128^3 grid, then eps = exp(-rho)*(EPS_OUT-EPS_IN)+EPS_IN).

Strategy (pure data-parallel over batch, 8 cores):
  - Each core owns one batch: its atoms + full 128^3 grid.
  - The (x, y) plane is tiled into 128 buckets of 8x16 voxels. Each atom's
    8x8x8 separable Gaussian window overlaps 1-4 buckets -> one "entry" per
    overlapped bucket. Every bucket's entries start at a fresh 128-partition
    tile (consecutive matmuls with different partition bases crash the device).
  - On device, per entry (batched broadcast-AP vector ops, fp32 intermediates):
      gz[e, z]   = exp(-((z - za) s)^2) masked to the 8-wide z window
      patch[e,p] = gx[e,dx] * gy[e,dy]   (masked 8x16 window patch)
    Window mask folded via t2 = max(t, (10 v)^2 - 1225), v = coord - center.
  - Dense z restricted to [zlo, zhi) covering all windows; constant z-strips
    are memset in the staging tile.
  - Per bucket, PE matmuls (fp32 operands): rho[p, z] = patch.T @ gz in PSUM;
    epilogue exp(-rho)*72.5 + 6.5 on ACT/DVE.
  - Output is written bucket-major (nonempty buckets first) so each 4-bucket
    group is ONE contiguous DMA and all empty buckets are a few big constant
    fills; the host inverts the permutation (pure layout transform).

kernel(**inputs) takes the FULL inputs and returns the FULL output.
"""
import sys
import numpy as np

sys.path.insert(0, "/opt/trn_rl_repo")

import concourse.bacc as bacc
import concourse.tile as tile
import concourse.mybir as mybir
from concourse.bass_utils import run_bass_kernel_spmd

F32 = mybir.dt.float32
F16 = mybir.dt.float16
I32 = mybir.dt.int32
AF = mybir.ActivationFunctionType
ALU = mybir.AluOpType

BOX = 128
W = 8
EPS_IN = 6.5
EPS_OUT = 79.0
LN_SCALE = float(np.log(EPS_OUT - EPS_IN))  # ln(72.5)

TX, TY = 8, 16
NBX, NBY = BOX // TX, BOX // TY     # 16, 8
NBUCKET = NBX * NBY                 # 128
DEAD_F = 1000.0
CUSHION = 2
NCORES = 8
MM_DT = F32   # PE fp32 (emulated): rel err 1.4e-5 vs 6.3e-4 for fp16, ~same time
PARAM_KEYS = ("ZA", "S", "CZ", "RX", "CX", "RY", "CY")
NPK = len(PARAM_KEYS)


# ----------------------------------------------------------------- host side

def _bucket_counts(coords_b, num_atoms_b):
    n = int(num_atoms_b)
    xyz = coords_b.reshape(-1, 3)[:n]
    base = np.floor(xyz).astype(np.int64) - (W // 2 - 1)
    bx, by = base[:, 0], base[:, 1]
    jx0, jx1 = bx // TX, (bx + W - 1) // TX
    jy0, jy1 = by // TY, (by + W - 1) // TY
    cnt = np.zeros(NBUCKET, np.int64)
    np.add.at(cnt, jx0 * NBY + jy0, 1)
    m = jx1 != jx0
    np.add.at(cnt, jx1[m] * NBY + jy0[m], 1)
    m2 = jy1 != jy0
    np.add.at(cnt, jx0[m2] * NBY + jy1[m2], 1)
    m3 = m & m2
    np.add.at(cnt, jx1[m3] * NBY + jy1[m3], 1)
    return cnt


def _make_layout(units):
    """Every bucket starts at a tile (128-partition) boundary; all matmul
    chunks contract partitions [0, plen)."""
    starts = np.zeros(NBUCKET, np.int64)
    tot = 0
    for b in range(NBUCKET):
        starts[b] = tot
        tot += 4 * ((int(units[b]) + 3) // 4)
    nt = max(1, (tot + 3) // 4)
    chunks_by_bucket = {}
    for b in range(NBUCKET):
        if units[b] == 0:
            continue
        t0 = int(starts[b]) // 4
        rem = int(units[b])
        segs = []
        t = t0
        while rem > 0:
            take = min(rem, 4)
            segs.append((t, 0, take * 32))
            t += 1
            rem -= take
        chunks_by_bucket[b] = segs
    return nt, starts, chunks_by_bucket


def _host_prep(coords_b, sigma_b, num_atoms_b, units, starts, nt):
    """Single [128, 7*nt] f32 param image (concatenated PARAM_KEYS)."""
    n = int(num_atoms_b)
    xyz = coords_b.reshape(-1, 3)[:n].astype(np.float64)
    sig = sigma_b[:n].astype(np.float64)
    xa, ya, za = xyz[:, 0], xyz[:, 1], xyz[:, 2]
    base = np.floor(xyz).astype(np.int64) - (W // 2 - 1)
    bx, by, bz = base[:, 0], base[:, 1], base[:, 2]
    s = np.sqrt(0.5) / sig
    assert (base >= 0).all() and (base + W <= BOX).all(), "window out of bounds"

    params = {k: np.zeros((128, nt), np.float32) for k in PARAM_KEYS}
    params["S"][:] = 1.0
    for k in ("CZ", "CX", "CY"):
        params[k][:] = -DEAD_F

    jx0, jx1 = bx // TX, (bx + W - 1) // TX
    jy0, jy1 = by // TY, (by + W - 1) // TY
    counts = np.zeros(NBUCKET, np.int64)
    for a in range(n):
        for jx in {int(jx0[a]), int(jx1[a])}:
            for jy in {int(jy0[a]), int(jy1[a])}:
                b = jx * NBY + jy
                i = counts[b]
                assert i < units[b] * 32, f"bucket {b} overflow"
                counts[b] += 1
                g = starts[b] * 32 + i
                t, p = g // 128, g % 128
                rx = xa[a] - TX * jx
                ry = ya[a] - TY * jy
                params["ZA"][p, t] = za[a]
                params["S"][p, t] = s[a]
                params["CZ"][p, t] = bz[a] + 3.5
                params["RX"][p, t] = rx
                params["CX"][p, t] = (bx[a] - TX * jx) + 3.5
                params["RY"][p, t] = ry
                params["CY"][p, t] = (by[a] - TY * jy) + 3.5
    return np.concatenate([params[k] for k in PARAM_KEYS], axis=1)


# --------------------------------------------------------------- device side

def _emit_build(nc, pool, iota_f, par, nt_c, c0, width, rkey, fkey,
                out_tile, out_cols, t2_pool=False):
    """Masked-Gaussian rows -> out_tile[:, out_cols] (fp16).

    par(key, c0, nt_c) -> broadcast AP. Engine split: d0/d/t2 on DVE,
    v on Pool(gpsimd), t/u/exp on ACT.
    """
    ncol = nt_c * width
    sh3 = [128, nt_c, width]

    def b_iota():
        return iota_f[:].rearrange("p (o w) -> p o w", o=1).broadcast_to(sh3)

    def r3(tl):
        return tl[:].rearrange("p (t w) -> p t w", w=width)

    d0 = pool.tile([128, ncol], F32, tag=f"d0_{width}")
    nc.vector.tensor_tensor(r3(d0), b_iota(), par(rkey, c0, nt_c),
                            op=ALU.subtract)
    d = pool.tile([128, ncol], F32, tag=f"d_{width}")
    nc.vector.tensor_tensor(r3(d), r3(d0), par("S", c0, nt_c), op=ALU.mult)
    t = pool.tile([128, ncol], F32, tag=f"t_{width}")
    nc.scalar.activation(t[:], d[:], AF.Square)
    v = pool.tile([128, ncol], F32, tag=f"v_{width}")
    nc.gpsimd.tensor_tensor(r3(v), b_iota(), par(fkey, c0, nt_c),
                            op=ALU.subtract)
    u = pool.tile([128, ncol], F32, tag=f"u_{width}")
    nc.scalar.activation(u[:], v[:], AF.Square, scale=10.0)
    t2 = pool.tile([128, ncol], F32, tag=f"t2_{width}")
    t2eng = nc.gpsimd if t2_pool else nc.vector
    t2eng.scalar_tensor_tensor(
        t2[:], u[:], -1225.0, t[:], op0=ALU.add, op1=ALU.max)
    nc.scalar.activation(out_tile[:, out_cols], t2[:], AF.Exp, scale=-1.0)


def _build_nc(nt, chunks_by_bucket, nonempty, empty, zlo, zhi):
    from contextlib import ExitStack
    zw = zhi - zlo
    nne = len(nonempty)
    nc = bacc.Bacc("TRN2", target_bir_lowering=False, debug=False,
                   num_devices=NCORES)
    param_d = nc.dram_tensor("PARAMS", [128, NPK * nt], F32,
                             kind="ExternalInput")
    eps_d = nc.dram_tensor("eps", [NBUCKET * 128 * BOX], F32,
                           kind="ExternalOutput")
    # bucket-major, rank order = nonempty + empty; [p, rank, z] iteration view
    eps_prz = eps_d.ap().rearrange("(b p z) -> p b z", b=NBUCKET, p=128, z=BOX)

    NCH = max(1, (nt + 5) // 6)
    bounds = [round(i * nt / NCH) for i in range(NCH + 1)]

    with tile.TileContext(nc) as tc, ExitStack() as ctx:
        const = ctx.enter_context(tc.tile_pool(name="const", bufs=1))
        big = ctx.enter_context(tc.tile_pool(name="big", bufs=1))
        scratch = ctx.enter_context(tc.tile_pool(name="scratch", bufs=5))
        xyscr = ctx.enter_context(tc.tile_pool(name="xyscr", bufs=1))
        psum = ctx.enter_context(tc.tile_pool(name="psum", bufs=6, space="PSUM"))
        epsp = ctx.enter_context(tc.tile_pool(name="epsp", bufs=6))

        # ---- constants
        iotas = {}
        for width, base in ((TX, 0), (TY, 0), (zw, zlo)):
            ii = const.tile([128, width], I32, tag=f"ii_{width}")
            nc.gpsimd.iota(ii[:], pattern=[[1, width]], base=base,
                           channel_multiplier=0)
            iof = const.tile([128, width], F32, tag=f"iof_{width}")
            nc.vector.tensor_copy(iof[:], ii[:])
            iotas[width] = iof
        c79 = const.tile([128, 2048], F32)
        nc.vector.memset(c79[:], EPS_OUT)
        bias_ln = const.tile([128, 1], F32)
        nc.vector.memset(bias_ln[:], LN_SCALE)

        # ---- params (one DMA)
        par_t = const.tile([128, NPK * nt], F32)
        nc.sync.dma_start(par_t[:], param_d.ap())
        pk_off = {k: i * nt for i, k in enumerate(PARAM_KEYS)}

        def mkpar(width):
            def par(key, c0, nt_c):
                o = pk_off[key] + c0
                return (par_t[:, o:o + nt_c]
                        .rearrange("p (t o) -> p t o", o=1)
                        .broadcast_to([128, nt_c, width]))
            return par

        # ---- persistent matmul operands (fp16)
        gz_t = big.tile([128, nt * zw], MM_DT, tag="gz")
        patch_t = big.tile([128, nt * TX * TY], MM_DT, tag="patch")
        gxm = big.tile([128, nt * TX], MM_DT, tag="gxm")
        gym = big.tile([128, nt * TY], MM_DT, tag="gym")

        # ---- constant fill for the empty-bucket tail (ranks nne..NBUCKET)
        FILLW = 16  # slots per fill DMA (c79 holds 16*128 elems per partition)
        r = nne
        while r < NBUCKET:
            rn = min(FILLW, NBUCKET - r)
            nc.sync.dma_start(eps_prz[:, r:r + rn, :],
                              c79[:, :rn * BOX]
                              .rearrange("p (b z) -> p b z", z=BOX))
            r += rn

        # ---- x/y window builds (small; unchunked)
        _emit_build(nc, xyscr, iotas[TX], mkpar(TX), nt, 0, TX,
                    "RX", "CX", gxm, slice(0, nt * TX))
        _emit_build(nc, xyscr, iotas[TY], mkpar(TY), nt, 0, TY,
                    "RY", "CY", gym, slice(0, nt * TY))

        # ---- per-group matmul + epilogue emitter
        def emit_group(grp, rank):
            ng = len(grp)
            acc = psum.tile([128, 512], F32)
            for q, b in enumerate(grp):
                oc = slice(q * BOX + zlo, q * BOX + zhi)
                nseg = len(chunks_by_bucket[b])
                for i, (t, plo, plen) in enumerate(chunks_by_bucket[b]):
                    nc.tensor.matmul(
                        acc[:, oc],
                        patch_t[plo:plo + plen, t * TX * TY:(t + 1) * TX * TY],
                        gz_t[plo:plo + plen, t * zw:(t + 1) * zw],
                        start=(i == 0), stop=(i == nseg - 1))
            ep = epsp.tile([128, 512], F32)
            epv = ep[:].rearrange("p (q z) -> p q z", z=BOX)
            accv = acc[:].rearrange("p (q z) -> p q z", z=BOX)
            if zlo > 0:
                nc.gpsimd.memset(epv[:, :ng, 0:zlo], EPS_OUT)
            if zhi < BOX:
                nc.gpsimd.memset(epv[:, :ng, zhi:BOX], EPS_OUT)
            nc.scalar.activation(epv[:, :ng, zlo:zhi], accv[:, :ng, zlo:zhi],
                                 AF.Exp, bias=bias_ln[:], scale=-1.0)
            nc.vector.tensor_scalar_add(epv[:, :ng, zlo:zhi],
                                        epv[:, :ng, zlo:zhi], EPS_IN)
            nc.sync.dma_start(eps_prz[:, rank:rank + ng, :],
                              epv[:, :ng, :])

        groups = [nonempty[i:i + 4] for i in range(0, len(nonempty), 4)]
        glast = [max(t + 1 for b in grp for (t, _, _) in chunks_by_bucket[b])
                 for grp in groups]
        granks = list(np.cumsum([0] + [len(g) for g in groups[:-1]]))

        # ---- z build + patch outer, chunked; groups interleave as their
        #      tiles complete
        gidx = 0
        for c in range(NCH):
            c0, c1 = bounds[c], bounds[c + 1]
            nt_c = c1 - c0
            if nt_c == 0:
                continue
            _emit_build(nc, scratch, iotas[zw], mkpar(zw), nt_c, c0, zw,
                        "ZA", "CZ", gz_t, slice(c0 * zw, c1 * zw))
            sh4 = [128, nt_c, TX, TY]
            gx_b = (gxm[:, c0 * TX:c1 * TX]
                    .rearrange("p (t x o) -> p t x o", x=TX, o=1)
                    .broadcast_to(sh4))
            gy_b = (gym[:, c0 * TY:c1 * TY]
                    .rearrange("p (t o y) -> p t o y", o=1, y=TY)
                    .broadcast_to(sh4))
            eng = nc.gpsimd if (c % 2 == 1) else nc.vector
            eng.tensor_tensor(
                patch_t[:, c0 * TX * TY:c1 * TX * TY]
                .rearrange("p (t x y) -> p t x y", x=TX, y=TY),
                gx_b, gy_b, op=ALU.mult)
            while gidx < len(groups) and glast[gidx] <= c1:
                emit_group(groups[gidx], int(granks[gidx]))
                gidx += 1
        while gidx < len(groups):
            emit_group(groups[gidx], int(granks[gidx]))
            gidx += 1
    nc.compile()
    return nc


# ------------------------------------------------------------------- driver

_CACHE = {}


def _layout_from_inputs(coords, num_atoms):
    B = coords.shape[0]
    all_counts = np.stack([
        _bucket_counts(coords[b], num_atoms[b]) for b in range(B)])
    mx = all_counts.max(axis=0)
    units = np.where(mx > 0, (mx + CUSHION + 31) // 32, 0)
    nt, starts, chunks_by_bucket = _make_layout(units)
    nonempty = [b for b in range(NBUCKET) if units[b] > 0]
    empty = [b for b in range(NBUCKET) if units[b] == 0]
    zlo, zhi = BOX, 0
    for b in range(B):
        n = int(num_atoms[b])
        z = coords[b].reshape(-1, 3)[:n, 2]
        bz = np.floor(z).astype(np.int64) - (W // 2 - 1)
        zlo = min(zlo, int(bz.min()))
        zhi = max(zhi, int(bz.max()) + W)
    zlo = max(0, (zlo // 4) * 4)
    zhi = min(BOX, ((zhi + 3) // 4) * 4)
    return units, nt, starts, chunks_by_bucket, nonempty, empty, zlo, zhi


def kernel(coords, assigned_params, num_atoms):
    coords = np.asarray(coords, dtype=np.float32)
    assigned_params = np.asarray(assigned_params, dtype=np.float32)
    num_atoms = np.asarray(num_atoms, dtype=np.int32)
    B = coords.shape[0]
    assert B == NCORES, f"expected {NCORES} batches, got {B}"

    (units, nt, starts, cbb, nonempty, empty, zlo,
     zhi) = _layout_from_inputs(coords, num_atoms)

    in_maps = []
    for b in range(B):
        img = _host_prep(coords[b], assigned_params[b, :, 1], num_atoms[b],
                         units, starts, nt)
        in_maps.append({"PARAMS": img})

    key = (tuple(units.tolist()), zlo, zhi)
    if key not in _CACHE:
        _CACHE[key] = _build_nc(nt, cbb, nonempty, empty, zlo, zhi)
    nc = _CACHE[key]
    res = run_bass_kernel_spmd(nc, in_maps, list(range(NCORES)))

    # invert the bucket-rank permutation (pure layout transform)
    order = nonempty + empty           # rank -> bucket id
    inv = np.empty(NBUCKET, np.int64)
    inv[np.array(order)] = np.arange(NBUCKET)
    out = np.empty((B, BOX, BOX, BOX), np.float32)
    for i in range(B):
        raw = res.results[i]["eps"].reshape(NBUCKET, 128, BOX)
        byb = raw[inv]                                 # [bucket, (dx dy), z]
        out[i] = (byb.reshape(NBX, NBY, TX, TY, BOX)
                  .transpose(0, 2, 1, 3, 4)
                  .reshape(BOX, BOX, BOX))
    return out


if __name__ == "__main__":
    rng = np.random.default_rng(0)
    coords = rng.uniform(0.2 * BOX, 0.8 * BOX, (8, 3 * 2048)).astype(np.float32)
    ap = np.stack([np.ones((8, 2048), np.float32),
                   1.0 + rng.random((8, 2048), dtype=np.float32)], axis=-1)
    na = np.full((8,), 2048, np.int32)
    out = kernel(coords=coords, assigned_params=ap, num_atoms=na)
    print("out", out.shape, out.dtype, out.min(), out.max())
